# revision 2
# baseline (speedup 1.0000x reference)
"""Trainium2 Bass kernel for the per-task (mixture-of-experts style) VAE.

Reference computation (B=4096 tokens, D=1024, H=2048, L=256, T=8 tasks):
every token belongs to one task; the reference runs all 8 per-task
encoders/heads on the full batch and masks.  Here we route instead:
core t processes exactly the tokens of task t (expert parallelism,
T == n_cores == 8), so each core runs ONE encoder/head stack on ~B/8
tokens.

Per-core device kernel: feature-major layout (features on SBUF
partitions, tokens on the free dimension).  All matmuls are bf16 with
fp32 PSUM accumulation; bias+activation fused on the scalar engine.
Host does the gather/pad/transpose + scatter (cheap numpy).
"""

import math

import numpy as np
import ml_dtypes

B, D, H, L, T = 4096, 1024, 2048, 256, 8
NCORES = 8
BF16 = ml_dtypes.bfloat16

# name, in_features, out_features, kind
LAYERS = [
    ("w1", D, H, "relu"),
    ("w2", H, H, "relu"),
    ("w3", H, H, "relu"),
    ("w4", H, 2 * L, "enc4"),
    ("dw1", L, H, "relu"),
    ("dw2", H, H, "relu"),
    ("h1", H, H, "relu"),
    ("h2", H, D, "out"),
]
NBIAS = sum(g // 128 for _, _, g, _ in LAYERS)  # 108 bias columns

_BUILD_CACHE: dict[int, dict] = {}


def _build(C: int) -> dict:
    """Build + compile the per-core Bass module for token capacity C."""
    if C in _BUILD_CACHE:
        return _BUILD_CACHE[C]

    import concourse.mybir as mybir
    from concourse import bacc
    from concourse.tile import TileContext

    f32 = mybir.dt.float32
    bf16 = mybir.dt.bfloat16
    Act = mybir.ActivationFunctionType

    ctiles = []
    c0 = 0
    while c0 < C:
        cw = min(512, C - c0)
        ctiles.append((c0, cw))
        c0 += cw

    nc = bacc.Bacc(None, target_bir_lowering=False, debug=False)

    xT = nc.dram_tensor("xT", [128, D // 128, C], bf16, kind="ExternalInput")
    epsT = nc.dram_tensor("epsT", [128, L // 128, C], f32, kind="ExternalInput")
    biases = nc.dram_tensor("biases", [128, NBIAS], f32, kind="ExternalInput")
    wdram = {
        name: nc.dram_tensor(name, [g // 128, 128, f], bf16, kind="ExternalInput")
        for name, f, g, _ in LAYERS
    }
    outT = nc.dram_tensor("outT", [128, D // 128, C], f32, kind="ExternalOutput")

    with TileContext(nc) as tc:
        with (
            tc.tile_pool(name="io", bufs=1) as io_pool,
            tc.tile_pool(name="act", bufs=2) as act_pool,
            tc.tile_pool(name="wp", bufs=3) as w_pool,
            tc.tile_pool(name="sm", bufs=1) as sm_pool,
            tc.tile_pool(name="op", bufs=3) as out_pool,
            tc.tile_pool(name="ps", bufs=6, space="PSUM") as ps_pool,
        ):
            xt = io_pool.tile([128, D // 128, C], bf16)
            nc.sync.dma_start(out=xt, in_=xT[:])
            ept = io_pool.tile([128, L // 128, C], f32)
            nc.sync.dma_start(out=ept, in_=epsT[:])
            bt = io_pool.tile([128, NBIAS], f32)
            nc.sync.dma_start(out=bt, in_=biases[:])

            cur = xt
            mu = ex = None
            boff = 0
            for name, f, g, kind in LAYERS:
                KT, GT = f // 128, g // 128
                if kind == "relu":
                    nxt = act_pool.tile([128, GT, C], bf16, tag="h")
                elif kind == "enc4":
                    mu = sm_pool.tile([128, L // 128, C], f32, tag="mu")
                    ex = sm_pool.tile([128, L // 128, C], f32, tag="ex")
                for gt in range(GT):
                    wt = w_pool.tile([128, KT, 128], bf16, tag="w")
                    nc.sync.dma_start(out=wt, in_=wdram[name][gt])
                    bias_ap = bt[:, boff + gt : boff + gt + 1]
                    if kind == "out":
                        ot = out_pool.tile([128, C], f32, tag="ot")
                    for c0, cw in ctiles:
                        ps = ps_pool.tile([128, 512], f32, tag="ps")
                        for kt in range(KT):
                            nc.tensor.matmul(
                                ps[:, :cw],
                                wt[:, kt, :],
                                cur[:, kt, c0 : c0 + cw],
                                start=(kt == 0),
                                stop=(kt == KT - 1),
                            )
                        if kind == "relu":
                            nc.scalar.activation(
                                nxt[:, gt, c0 : c0 + cw], ps[:, :cw],
                                Act.Relu, bias=bias_ap,
                            )
                        elif kind == "enc4":
                            if gt < L // 128:
                                nc.scalar.activation(
                                    mu[:, gt, c0 : c0 + cw], ps[:, :cw],
                                    Act.Identity, bias=bias_ap,
                                )
                            else:
                                nc.scalar.activation(
                                    ex[:, gt - L // 128, c0 : c0 + cw], ps[:, :cw],
                                    Act.Exp, bias=bias_ap,
                                )
                        elif kind == "out":
                            nc.scalar.activation(
                                ot[:, c0 : c0 + cw], ps[:, :cw],
                                Act.Sigmoid, bias=bias_ap,
                            )
                    if kind == "out":
                        nc.sync.dma_start(out=outT[:, gt, :], in_=ot)
                boff += GT
                if kind == "relu":
                    cur = nxt
                elif kind == "enc4":
                    # z = mu + exp(log_sigma) * eps   (reparameterization)
                    zt = sm_pool.tile([128, L // 128, C], bf16, tag="z")
                    for j in range(L // 128):
                        tmp = sm_pool.tile([128, C], f32, tag=f"tmp{j}")
                        nc.vector.tensor_mul(tmp, ex[:, j], ept[:, j])
                        nc.vector.tensor_add(zt[:, j], tmp, mu[:, j])
                    cur = zt

    nc.compile()
    meta = {"nc": nc, "C": C}
    _BUILD_CACHE[C] = meta
    return meta


_EXEC_CACHE: dict[int, tuple] = {}


def _executor(C: int):
    """Sharded 8-core jitted executor for capacity C (built once)."""
    if C in _EXEC_CACHE:
        return _EXEC_CACHE[C]

    import jax
    import numpy as jnp_np  # noqa
    from jax.sharding import Mesh, PartitionSpec
    from jax.experimental.shard_map import shard_map
    import concourse.mybir as mybir
    from concourse.bass2jax import (
        _bass_exec_p,
        install_neuronx_cc_hook,
        partition_id_tensor,
    )

    meta = _build(C)
    nc = meta["nc"]
    install_neuronx_cc_hook()

    partition_name = nc.partition_id_tensor.name if nc.partition_id_tensor else None
    in_names, out_names, out_avals, zero_shapes = [], [], [], []
    for alloc in nc.m.functions[0].allocations:
        if not isinstance(alloc, mybir.MemoryLocationSet):
            continue
        name = alloc.memorylocations[0].name
        if alloc.kind == "ExternalInput":
            if name != partition_name:
                in_names.append(name)
        elif alloc.kind == "ExternalOutput":
            shape = tuple(alloc.tensor_shape)
            dtype = mybir.dt.np(alloc.dtype)
            out_names.append(name)
            out_avals.append(jax.core.ShapedArray(shape, dtype))
            zero_shapes.append((shape, dtype))
    n_params = len(in_names)
    n_outs = len(out_names)
    all_in_names = list(in_names) + list(out_names)
    if partition_name is not None:
        all_in_names.append(partition_name)

    def _body(*args):
        operands = list(args)
        if partition_name is not None:
            operands.append(partition_id_tensor())
        outs = _bass_exec_p.bind(
            *operands,
            out_avals=tuple(out_avals),
            in_names=tuple(all_in_names),
            out_names=tuple(out_names),
            lowering_input_output_aliases=(),
            sim_require_finite=True,
            sim_require_nnan=True,
            nc=nc,
        )
        return tuple(outs)

    devices = jax.devices()[:NCORES]
    mesh = Mesh(np.asarray(devices), ("core",))
    in_specs = (PartitionSpec("core"),) * (n_params + n_outs)
    out_specs = (PartitionSpec("core"),) * n_outs
    donate = tuple(range(n_params, n_params + n_outs))
    sharded = jax.jit(
        shard_map(_body, mesh=mesh, in_specs=in_specs, out_specs=out_specs,
                  check_rep=False),
        donate_argnums=donate,
        keep_unused=True,
    )
    entry = (sharded, in_names, out_names, out_avals, zero_shapes)
    _EXEC_CACHE[C] = entry
    return entry


def run_cores(C: int, in_maps: list[dict[str, np.ndarray]]) -> list[np.ndarray]:
    """Run the compiled kernel on 8 cores; returns per-core outT arrays."""
    sharded, in_names, out_names, out_avals, zero_shapes = _executor(C)
    concat_in = [
        np.concatenate([in_maps[c][name] for c in range(NCORES)], axis=0)
        for name in in_names
    ]
    concat_zeros = [
        np.zeros((NCORES * s[0], *s[1:]), dt) for s, dt in zero_shapes
    ]
    out_arrs = sharded(*concat_in, *concat_zeros)
    out = np.asarray(out_arrs[0])
    per_core_shape = out_avals[0].shape
    return [
        out.reshape(NCORES, *per_core_shape)[c] for c in range(NCORES)
    ]


def _tile_weight(w: np.ndarray) -> np.ndarray:
    """[F, G] -> [G/128, 128(k-in-tile), F] bf16, matching the SBUF tile
    layout [partition=k, kt, g] flattened per out-feature tile."""
    f, g = w.shape
    return np.ascontiguousarray(
        w.reshape(f // 128, 128, g // 128, 128).transpose(2, 1, 0, 3)
        .reshape(g // 128, 128, f)
    ).astype(BF16)


def _tile_tokens(a: np.ndarray, C: int, dtype) -> np.ndarray:
    """[n, F] token-major -> [128, F/128, C] feature-major, zero-padded."""
    n, f = a.shape
    pad = np.zeros((C, f), np.float32)
    pad[:n] = a
    return np.ascontiguousarray(
        pad.T.reshape(f // 128, 128, C).transpose(1, 0, 2)
    ).astype(dtype)


def kernel(**inputs: np.ndarray) -> np.ndarray:
    x = np.asarray(inputs["x"], np.float32)
    task = np.asarray(inputs["task"]).astype(np.int64)
    eps = np.asarray(inputs["eps"], np.float32)

    order = np.argsort(task, kind="stable")
    counts = np.bincount(task, minlength=T)
    idx_by_task = np.split(order, np.cumsum(counts)[:-1])
    max_count = int(counts.max())

    rounds = max(1, math.ceil(max_count / 1024))
    per_round = math.ceil(max_count / rounds)
    C = max(512, ((per_round + 127) // 128) * 128)

    # per-core constant tensors (weights + biases)
    bias_blocks = []
    wmaps = []
    for t in range(T):
        wm = {
            "w1": _tile_weight(np.asarray(inputs["enc_W1"][t], np.float32)),
            "w2": _tile_weight(np.asarray(inputs["enc_W2"][t], np.float32)),
            "w3": _tile_weight(np.asarray(inputs["enc_W3"][t], np.float32)),
            "w4": _tile_weight(np.asarray(inputs["enc_W4"][t], np.float32)),
            "h1": _tile_weight(np.asarray(inputs["hd_W1"][t], np.float32)),
            "h2": _tile_weight(np.asarray(inputs["hd_W2"][t], np.float32)),
        }
        if t == 0:
            wm["dw1"] = _tile_weight(np.asarray(inputs["ds_W1"], np.float32))
            wm["dw2"] = _tile_weight(np.asarray(inputs["ds_W2"], np.float32))
        else:
            wm["dw1"] = wmaps[0]["dw1"]
            wm["dw2"] = wmaps[0]["dw2"]
        wmaps.append(wm)
        bs = [
            np.asarray(inputs["enc_b1"][t], np.float32),
            np.asarray(inputs["enc_b2"][t], np.float32),
            np.asarray(inputs["enc_b3"][t], np.float32),
            np.asarray(inputs["enc_b4"][t], np.float32),
            np.asarray(inputs["ds_b1"], np.float32),
            np.asarray(inputs["ds_b2"], np.float32),
            np.asarray(inputs["hd_b1"][t], np.float32),
            np.asarray(inputs["hd_b2"][t], np.float32),
        ]
        bias_blocks.append(
            np.concatenate(
                [b.reshape(-1, 128).T for b in bs], axis=1
            ).astype(np.float32)
        )

    out = np.empty((B, D), np.float32)
    for r in range(rounds):
        in_maps = []
        round_idx = []
        for t in range(T):
            idx = idx_by_task[t][r * C : (r + 1) * C]
            round_idx.append(idx)
            m = dict(wmaps[t])
            m["biases"] = bias_blocks[t]
            m["xT"] = _tile_tokens(x[idx], C, BF16)
            m["epsT"] = _tile_tokens(eps[idx], C, np.float32)
            in_maps.append(m)
        results = run_cores(C, in_maps)
        for t in range(T):
            idx = round_idx[t]
            if len(idx) == 0:
                continue
            # [128, D/128, C] -> [D, C] -> tokens [count, D]
            yT = results[t].transpose(1, 0, 2).reshape(D, C)
            out[idx] = yT[:, : len(idx)].T
    return out
